# revision 42
# baseline (speedup 1.0000x reference)
"""Trainium2 Bass kernel for nn_EquivariantModel (e3nn-style equivariant net).

Strategy: data-parallel over batch (8 cores x 1024 rows), feature-major
activations.  All o3.Linear layers (l1/l2, block out-linears, final) are
folded host-side into the FullyConnectedTensorProduct weights, so each block
reduces to a bilinear form in its RAW inputs:

    tp_s[b,w] = sum_{pq} s_p s_q MSS[pq,w] + sum_{i,pq} v_ip v_iq MVV[pq,w]
    tp_v[b,w,i] = sum_{pq} s_p v_iq MX[pq,w]

The symmetric forms (s(x)s, v(x)v) need only cyclic diagonals d=0..M/2 (2x
fewer products); products z_d = x * rot_d(x) are built with single
full-width DVE multiplies against partition-rotated copies of x, which are
materialized by grouped DMA reads from a row-doubled DRAM image (one DMA
covers many rotations via an overlapping-stride access pattern; the three
vector components share one image so a group is a single DMA).  Matmuls
stream z through per-diagonal weight tiles, accumulating in PSUM; the ss,
vv and sv streams are interleaved in one d-loop to keep DMA demand flat.
Block 1 (mult 64) stacks two batch halves in the partition dim and uses
split-K matmuls at base partitions 0/64.  A fraction of the products runs
on the GpSimd engine to unload the DVE.
"""

import sys
import numpy as np

if '/opt/trn_rl_repo' not in sys.path:
    sys.path.insert(0, '/opt/trn_rl_repo')

B, M_IN, M_HID = 8192, 64, 128
N_CORES = 8
BC = B // N_CORES            # batch per core
TANH_GAIN = 1.5927116870880127

GRP = 8                      # s-rotations / weight tiles per group DMA
GRPV = 2                     # b2 v-rotation group size (diagonals)
GRPV1 = 1                    # b1 v-rotation group size (pairs)
POOL_EVERY = 5               # every Nth product goes to GpSimd instead of DVE

_CACHE = {}


def _build_program(repeat=1):
    import concourse.mybir as mybir
    import concourse.tile as tile
    from concourse import bacc
    from contextlib import ExitStack
    import bass_rust

    f16 = mybir.dt.float16
    f32 = mybir.dt.float32

    nc = bacc.Bacc("TRN2", target_bir_lowering=False)

    # ---- DRAM I/O ----
    s2d = nc.dram_tensor("s2d", [128, BC], f16, kind="ExternalInput")
    v2da = nc.dram_tensor("v2da", [128, 3 * BC], f16, kind="ExternalInput")
    wss1 = nc.dram_tensor("wss1", [128, 17, 128], f16, kind="ExternalInput")
    wvv1 = nc.dram_tensor("wvv1", [128, 17, 128], f16, kind="ExternalInput")
    wsv1 = nc.dram_tensor("wsv1", [128, 32, 128], f16, kind="ExternalInput")
    wss2 = nc.dram_tensor("wss2", [128, 65, 128], f16, kind="ExternalInput")
    wvv2 = nc.dram_tensor("wvv2", [128, 65, 128], f16, kind="ExternalInput")
    wsv2 = nc.dram_tensor("wsv2", [128, 128, 128], f16, kind="ExternalInput")
    gate_w = {}
    for blk in ("1", "2"):
        for nm in ("s", "g", "v"):
            gate_w[blk + nm] = nc.dram_tensor(f"g{blk}{nm}", [128, 128], f16,
                                              kind="ExternalInput")
    wfs = nc.dram_tensor("wfs", [128, 64], f16, kind="ExternalInput")
    wfv = nc.dram_tensor("wfv", [128, 64], f16, kind="ExternalInput")

    dts = nc.dram_tensor("dts", [256, BC], f16, kind="Internal")
    dgva = nc.dram_tensor("dgva", [256, 3 * BC], f16, kind="Internal")
    out_d = nc.dram_tensor("out", [256, BC], f32, kind="ExternalOutput")

    def src_ap(t, dims, offset):
        s = t[:].copy()
        s.ap = bass_rust.VecI64Pair(dims)
        s.offset = offset
        return s

    with ExitStack() as ctx:
        tc = ctx.enter_context(tile.TileContext(nc))
        consts = ctx.enter_context(tc.tile_pool(name="consts", bufs=1))
        acts = ctx.enter_context(tc.tile_pool(name="acts", bufs=1))
        rotp = ctx.enter_context(tc.tile_pool(name="rot", bufs=2))
        rotv = ctx.enter_context(tc.tile_pool(name="rotv", bufs=3))
        wstr = ctx.enter_context(tc.tile_pool(name="wstr", bufs=2))
        wstv = ctx.enter_context(tc.tile_pool(name="wstv", bufs=3))
        zp1 = ctx.enter_context(tc.tile_pool(name="z1p", bufs=6))
        zp2 = ctx.enter_context(tc.tile_pool(name="z2p", bufs=6))
        psp = ctx.enter_context(tc.tile_pool(name="ps", bufs=1, space="PSUM"))
        tmp = ctx.enter_context(tc.tile_pool(name="tmp", bufs=1))

        GW = {}
        for k, t in gate_w.items():
            w = consts.tile([128, 128], f16, tag=f"gw{k}", name=f"gw{k}")
            nc.sync.dma_start(w[:], t[:])
            GW[k] = w
        wfs_sb = consts.tile([128, 64], f16, tag="wfs", name="wfs")
        nc.sync.dma_start(wfs_sb[:], wfs[:])
        wfv_sb = consts.tile([128, 64], f16, tag="wfv", name="wfv")
        nc.sync.dma_start(wfv_sb[:], wfv[:])

        nmul = [0]

        def mul(z, a, b):
            nmul[0] += 1
            if POOL_EVERY and nmul[0] % POOL_EVERY == 0:
                nc.gpsimd.tensor_mul(z, a, b)
            else:
                nc.vector.tensor_mul(z, a, b)

        def b1_block():
            # bases: features duplicated across both partition halves
            sb = acts.tile([128, BC], f16, tag="sb1", name="sb1")
            nc.sync.dma_start(sb[0:64, :], s2d[0:64, :])
            nc.sync.dma_start(sb[64:128, :], s2d[0:64, :])
            vb = []
            for i in range(3):
                t = acts.tile([128, BC], f16, tag=f"vb1{i}", name=f"vb1{i}")
                nc.sync.dma_start(t[0:64, :], v2da[0:64, i * BC:(i + 1) * BC])
                nc.sync.dma_start(t[64:128, :], v2da[0:64, i * BC:(i + 1) * BC])
                vb.append(t)

            accs = psp.tile([128, 1024], f32, tag="pa_s", name="pa_s")
            accv = [psp.tile([128, 1024], f32, tag=f"pa_v{i}", name=f"pa_v{i}")
                    for i in range(3)]

            vrot = [None]
            wvv_t = [None]
            NPR = GRP // 2          # diagonal pairs per s-rot group

            # pair (2k, 2k+1): rot tile rows 0:64 = rot_2k, 64:128 = rot_2k+1
            for g0 in range(0, 64, GRP):
                npr = min(NPR, (64 - g0) // 2)
                rot = rotp.tile([128, NPR * BC], f16, tag="rotS1", name="rotS1")
                for h in range(2):
                    nc.sync.dma_start(
                        rot[h * 64:(h + 1) * 64, 0:npr * BC],
                        src_ap(s2d, [[BC, 64], [2 * BC, npr], [1, BC]],
                               (g0 + h) * BC))
                wsv_t = wstr.tile([128, NPR * 128], f16, tag="wsv", name="wsv")
                nc.scalar.dma_start(
                    wsv_t[:, 0:npr * 128],
                    wsv1[:, g0 // 2:g0 // 2 + npr, :].rearrange(
                        "p n m -> p (n m)"))
                nss = max(0, min(npr, 17 - g0 // 2))
                if nss > 0:
                    wss_t = wstr.tile([128, NPR * 128], f16, tag="wss", name="wss")
                    nc.scalar.dma_start(
                        wss_t[:, 0:nss * 128],
                        wss1[:, g0 // 2:g0 // 2 + nss, :].rearrange(
                            "p n m -> p (n m)"))
                for kp in range(npr):
                    gp = g0 // 2 + kp
                    rj = rot[:, kp * BC:(kp + 1) * BC]
                    for i in range(3):
                        z = zp1.tile([128, BC], f16, tag="z1", name="z1")
                        mul(z, rj, vb[i])
                        for h in range(2):
                            hs = slice(h * 512, (h + 1) * 512)
                            nc.tensor.matmul(
                                accv[i][:, hs],
                                wsv_t[:, kp * 128:(kp + 1) * 128], z[:, hs],
                                start=(gp == 0), stop=(gp == 31))
                    if gp > 16:
                        continue
                    z = zp1.tile([128, BC], f16, tag="z1", name="z1")
                    mul(z, sb, rj)
                    for h in range(2):
                        hs = slice(h * 512, (h + 1) * 512)
                        nc.tensor.matmul(accs[:, hs],
                                         wss_t[:, kp * 128:(kp + 1) * 128],
                                         z[:, hs],
                                         start=(gp == 0), stop=False)
                    # interleaved vv stream (pairs x 3 components)
                    vr = rotv.tile([128, 3 * BC], f16, tag="rotV1",
                                   name="rotV1")
                    for hh in range(2):
                        nc.sync.dma_start(
                            vr[hh * 64:(hh + 1) * 64, :],
                            src_ap(v2da,
                                   [[3 * BC, 64], [BC, 3], [1, BC]],
                                   (2 * gp + hh) * 3 * BC))
                    wt = wstv.tile([128, 128], f16, tag="wvv", name="wvv")
                    nc.scalar.dma_start(
                        wt[:, :],
                        wvv1[:, gp:gp + 1, :].rearrange("p n m -> p (n m)"))
                    for i in range(3):
                        z = zp1.tile([128, BC], f16, tag="z1", name="z1")
                        mul(z, vb[i], vr[:, i * BC:(i + 1) * BC])
                        for h in range(2):
                            hs = slice(h * 512, (h + 1) * 512)
                            nc.tensor.matmul(accs[:, hs], wt[:, :], z[:, hs],
                                             start=False,
                                             stop=(gp == 16 and i == 2))
            return accs, accv

        def b2_block(tanh_s, gated):
            # tanh_s / gated[i]: [128, 2*BC], cols BC:2BC duplicate 0:BC
            accs = psp.tile([128, 1024], f32, tag="pa_s", name="pa_s")
            accv = [psp.tile([128, 1024], f32, tag=f"pa_v{i}", name=f"pa_v{i}")
                    for i in range(3)]

            for g0 in range(0, 128, GRP):
                ng = min(GRP, 128 - g0)
                rot = rotp.tile([128, GRP * 1024], f16, tag="rotS2", name="rotS2")
                nc.sync.dma_start(
                    rot[:, 0:ng * 1024],
                    src_ap(dts, [[BC, 128], [BC, ng], [1, 1024]], g0 * BC))
                wsv_t = wstr.tile([128, GRP * 128], f16, tag="wsv", name="wsv")
                nc.scalar.dma_start(
                    wsv_t[:, 0:ng * 128],
                    wsv2[:, g0:g0 + ng, :].rearrange("p n m -> p (n m)"))
                nss = max(0, min(ng, 65 - g0))
                if nss > 0:
                    wss_t = wstr.tile([128, GRP * 128], f16, tag="wss", name="wss")
                    nc.scalar.dma_start(
                        wss_t[:, 0:nss * 128],
                        wss2[:, g0:g0 + nss, :].rearrange("p n m -> p (n m)"))
                for kp in range(ng // 2):
                    d0 = g0 + 2 * kp
                    for i in range(3):
                        z = zp2.tile([128, 2 * BC], f16, tag="z2", name="z2")
                        mul(z, gated[i], rot[:, 2 * kp * BC:(2 * kp + 2) * BC])
                        for jd in range(2):
                            d = d0 + jd
                            for h in range(2):
                                hs = slice(h * 512, (h + 1) * 512)
                                nc.tensor.matmul(
                                    accv[i][:, hs],
                                    wsv_t[:, (2 * kp + jd) * 128:
                                          (2 * kp + jd + 1) * 128],
                                    z[:, jd * BC + h * 512:
                                      jd * BC + (h + 1) * 512],
                                    start=(d == 0), stop=(d == 127))
                    if d0 > 64:
                        continue
                    if d0 < 64:
                        # ss pair
                        z = zp2.tile([128, 2 * BC], f16, tag="z2", name="z2")
                        mul(z, tanh_s, rot[:, 2 * kp * BC:(2 * kp + 2) * BC])
                        for jd in range(2):
                            for h in range(2):
                                hs = slice(h * 512, (h + 1) * 512)
                                nc.tensor.matmul(
                                    accs[:, hs],
                                    wss_t[:, (2 * kp + jd) * 128:
                                          (2 * kp + jd + 1) * 128],
                                    z[:, jd * BC + h * 512:
                                      jd * BC + (h + 1) * 512],
                                    start=(d0 + jd == 0), stop=False)
                        # vv pair (i-major pair layout in one group DMA)
                        vr = rotv.tile([128, 2 * 3 * BC], f16, tag="rotV2",
                                       name="rotV2")
                        nc.sync.dma_start(
                            vr[:, :],
                            src_ap(dgva, [[3 * BC, 128], [BC, 3], [3 * BC, 2],
                                          [1, BC]], d0 * 3 * BC))
                        wt = wstv.tile([128, 2 * 128], f16, tag="wvv",
                                       name="wvv")
                        nc.scalar.dma_start(
                            wt[:, :],
                            wvv2[:, d0:d0 + 2, :].rearrange("p n m -> p (n m)"))
                        for i in range(3):
                            z = zp2.tile([128, 2 * BC], f16, tag="z2", name="z2")
                            mul(z, gated[i], vr[:, i * 2 * BC:(i + 1) * 2 * BC])
                            for jd in range(2):
                                for h in range(2):
                                    hs = slice(h * 512, (h + 1) * 512)
                                    nc.tensor.matmul(
                                        accs[:, hs],
                                        wt[:, jd * 128:(jd + 1) * 128],
                                        z[:, jd * BC + h * 512:
                                          jd * BC + (h + 1) * 512],
                                        start=False, stop=False)
                    else:
                        # d0 == 64: single leftover diagonal for ss and vv
                        z = zp2.tile([128, 2 * BC], f16, tag="z2", name="z2")
                        mul(z[:, 0:BC], tanh_s[:, 0:BC],
                            rot[:, 2 * kp * BC:2 * kp * BC + BC])
                        for h in range(2):
                            hs = slice(h * 512, (h + 1) * 512)
                            nc.tensor.matmul(
                                accs[:, hs],
                                wss_t[:, 2 * kp * 128:(2 * kp + 1) * 128],
                                z[:, h * 512:(h + 1) * 512],
                                start=False, stop=False)
                        vr = rotv.tile([128, 2 * 3 * BC], f16, tag="rotV2",
                                       name="rotV2")
                        nc.sync.dma_start(
                            vr[:, 0:3 * BC],
                            src_ap(dgva, [[3 * BC, 128], [BC, 3], [1, BC]],
                                   64 * 3 * BC))
                        wt = wstv.tile([128, 2 * 128], f16, tag="wvv",
                                       name="wvv")
                        nc.scalar.dma_start(
                            wt[:, 0:128],
                            wvv2[:, 64:65, :].rearrange("p n m -> p (n m)"))
                        for i in range(3):
                            z = zp2.tile([128, 2 * BC], f16, tag="z2", name="z2")
                            mul(z[:, 0:BC], gated[i][:, 0:BC],
                                vr[:, i * BC:(i + 1) * BC])
                            for h in range(2):
                                hs = slice(h * 512, (h + 1) * 512)
                                nc.tensor.matmul(
                                    accs[:, hs], wt[:, 0:128],
                                    z[:, h * 512:(h + 1) * 512],
                                    start=False,
                                    stop=(i == 2))
            return accs, accv

        def gate(blk, accs, accv, dup=False):
            """PSUM accs -> (tanh_s, gated_v[3]) f16; dup doubles the free dim
            (cols BC:2BC replicate 0:BC) for paired-diagonal consumption."""
            W_ = 2 * BC if dup else BC
            tp_s = acts.tile([128, BC], f16, tag="tps", name="tps")
            nc.scalar.copy(tp_s[:, :], accs[:, :])
            tp_v = []
            for i in range(3):
                t = acts.tile([128, BC], f16, tag=f"tpv{i}", name=f"tpv{i}")
                nc.scalar.copy(t[:, :], accv[i][:, :])
                tp_v.append(t)
            tanh_s = acts.tile([128, W_], f16, tag=f"ths{blk}", name=f"ths{blk}")
            tg = acts.tile([128, BC], f16, tag="tg", name="tg")
            vl = [acts.tile([128, BC], f16, tag=f"vl{i}", name=f"vl{i}")
                  for i in range(3)]
            # gate matmuls reuse the freed accumulator PSUM slots
            ps = psp.tile([128, 1024], f32, tag="pa_s", name="pa_s")
            psg = psp.tile([128, 1024], f32, tag="pa_v0", name="pa_v0")
            psv = [psp.tile([128, 1024], f32, tag=f"pa_v{i}", name=f"pa_v{i}")
                   for i in (1, 2)]
            psv.append(psp.tile([128, 1024], f32, tag="pa_s", name="pa_s"))
            for h in range(2):
                sl_ = slice(h * 512, (h + 1) * 512)
                nc.tensor.matmul(ps[:, sl_], GW[blk + "s"], tp_s[:, sl_],
                                 start=True, stop=True)
                nc.scalar.activation(tanh_s[:, sl_], ps[:, sl_],
                                     mybir.ActivationFunctionType.Tanh)
                nc.tensor.matmul(psg[:, sl_], GW[blk + "g"], tp_s[:, sl_],
                                 start=True, stop=True)
                nc.scalar.activation(tg[:, sl_], psg[:, sl_],
                                     mybir.ActivationFunctionType.Tanh)
                for i in range(3):
                    nc.tensor.matmul(psv[i][:, sl_], GW[blk + "v"],
                                     tp_v[i][:, sl_], start=True, stop=True)
                    nc.scalar.copy(vl[i][:, sl_], psv[i][:, sl_])
            gated = []
            for i in range(3):
                t = acts.tile([128, W_], f16, tag=f"gv{blk}{i}", name=f"gv{blk}{i}")
                nc.vector.tensor_mul(t[:, 0:BC], tg, vl[i])
                if dup:
                    nc.scalar.copy(t[:, BC:2 * BC], t[:, 0:BC])
                gated.append(t)
            if dup:
                nc.scalar.copy(tanh_s[:, BC:2 * BC], tanh_s[:, 0:BC])
            return tanh_s, gated

        def _network():
            accs, accv = b1_block()
            tanh_s1, gated1 = gate("1", accs, accv, dup=True)
            # doubled DRAM images for b2 rotations
            nc.sync.dma_start(dts[0:128, :], tanh_s1[:, 0:BC])
            nc.sync.dma_start(dts[128:256, :], tanh_s1[:, 0:BC])
            for i in range(3):
                nc.sync.dma_start(dgva[0:128, i * BC:(i + 1) * BC],
                                  gated1[i][:, 0:BC])
                nc.sync.dma_start(dgva[128:256, i * BC:(i + 1) * BC],
                                  gated1[i][:, 0:BC])
            accs2, accv2 = b2_block(tanh_s1, gated1)
            tanh_s2, gated2 = gate("2", accs2, accv2)
            # final linears (out-linears folded in)
            fps = psp.tile([128, 1024], f32, tag="pa_v0", name="pa_v0")
            fpv = [psp.tile([128, 1024], f32, tag=t, name=t)
                   for t in ("pa_v1", "pa_v2", "pa_s")]
            for h in range(2):
                sl_ = slice(h * 512, (h + 1) * 512)
                nc.tensor.matmul(fps[0:64, sl_], wfs_sb[:], tanh_s2[:, sl_],
                                 start=True, stop=True)
                ot = tmp.tile([64, 512], f32, tag="outs", name="outs")
                nc.scalar.copy(ot[:, :], fps[0:64, sl_])
                nc.sync.dma_start(out_d[0:64, sl_], ot[:])
                for i in range(3):
                    nc.tensor.matmul(fpv[i][0:64, sl_], wfv_sb[:],
                                     gated2[i][:, sl_], start=True, stop=True)
                    ov = tmp.tile([64, 512], f32, tag=f"outv{i}", name=f"outv{i}")
                    nc.scalar.copy(ov[:, :], fpv[i][0:64, sl_])
                    nc.sync.dma_start(out_d[64 + 64 * i:128 + 64 * i, sl_], ov[:])

        if repeat > 1:
            with tc.For_i(0, repeat, 1):
                _network()
        else:
            _network()

    nc.finalize()
    return nc


def _host_prep(inputs):
    """Fold all linears into TP weights; build diagonal weight streams."""
    f = {k: np.asarray(v, np.float64) for k, v in inputs.items() if k != 'x'}
    d = {}
    fold = {}
    for blk, M in (("b1", 64), ("b2", 128)):
        c1 = 1.0 / np.sqrt(M)
        A, Av = f[f"{blk}_l1_w0"] * c1, f[f"{blk}_l1_w1"] * c1
        Bm, Bv = f[f"{blk}_l2_w0"] * c1, f[f"{blk}_l2_w1"] * c1
        if blk == "b2":
            A, Av = fold["O1s"] @ A, fold["O1v"] @ Av
            Bm, Bv = fold["O1s"] @ Bm, fold["O1v"] @ Bv
        ctp = 1.0 / (M * np.sqrt(2.0))

        def fld(L, R, W, c):
            T = np.tensordot(L, W, axes=(1, 0))
            T = np.tensordot(R, T, axes=(1, 1))
            return c * T.transpose(1, 0, 2)
        MSS = fld(A, Bm, f[f"{blk}_tp_ss"], ctp)
        MVV = fld(Av, Bv, f[f"{blk}_tp_vv"], ctp / np.sqrt(3.0))
        MSV = fld(A, Bv, f[f"{blk}_tp_sv"], ctp)
        MVS = fld(Av, Bm, f[f"{blk}_tp_vs"], ctp)
        MX = MSV + MVS.transpose(1, 0, 2)
        Min = A.shape[0]
        ar = np.arange(Min)
        sym = {}
        for nm, Msym in (("ss", MSS), ("vv", MVV)):
            tiles = []
            for dd in range(Min // 2 + 1):
                idx = (ar + dd) % Min
                if dd == 0:
                    w = Msym[ar, ar, :]
                elif dd == Min // 2:
                    w = (Msym[ar, idx, :] + Msym[idx, ar, :]) * 0.5
                else:
                    w = Msym[ar, idx, :] + Msym[idx, ar, :]
                tiles.append(w)
            sym[nm] = np.stack(tiles, axis=1)                 # [Min, nd, 128]
        rect = np.stack([MX[(ar + c) % Min, ar, :] for c in range(Min)],
                        axis=1)                               # [Min, Min, 128]
        if blk == "b1":
            # pack diagonal pairs (2k, 2k+1) into 128-row tiles; odd counts
            # get a zero-padded bottom half
            def pairs(st):
                nd = st.shape[1]
                tiles = []
                for k in range((nd + 1) // 2):
                    top = st[:, 2 * k, :]
                    bot = (st[:, 2 * k + 1, :] if 2 * k + 1 < nd
                           else np.zeros_like(top))
                    tiles.append(np.concatenate([top, bot], axis=0))
                return np.stack(tiles, axis=1)                # [128, np, 128]
            for nm in ("ss", "vv"):
                sym[nm] = pairs(sym[nm])
            rect = pairs(rect)
        bn = blk[1]
        d[f"wss{bn}"] = np.ascontiguousarray(sym["ss"]).astype(np.float16)
        d[f"wvv{bn}"] = np.ascontiguousarray(sym["vv"]).astype(np.float16)
        d[f"wsv{bn}"] = np.ascontiguousarray(rect).astype(np.float16)
        cg = 1.0 / np.sqrt(128)
        for nm, sfx in (("ws", "s"), ("wg", "g"), ("wv", "v")):
            d[f"g{bn}{sfx}"] = (f[f"{blk}_g_{nm}"] * cg).astype(np.float16)
        cog = TANH_GAIN / np.sqrt(128)
        fold[f"O{bn}s"] = f[f"{blk}_o_w0"] * cog
        fold[f"O{bn}v"] = f[f"{blk}_o_w1"] * cog
    cf = 1.0 / np.sqrt(128)
    d["wfs"] = (fold["O2s"] @ (f["f_w0"] * cf)).astype(np.float16)
    d["wfv"] = (fold["O2v"] @ (f["f_w1"] * cf)).astype(np.float16)
    return d


def _make_in_maps(x, w):
    x = np.asarray(x, dtype=np.float32)
    in_maps = []
    for c in range(N_CORES):
        bs = slice(c * BC, (c + 1) * BC)
        xl = x[bs]
        s_loc = np.ascontiguousarray(xl[:, :64].T).astype(np.float16)   # [64, BC]
        v_loc = xl[:, 64:].reshape(BC, 64, 3)
        m = dict(w)
        m["s2d"] = np.concatenate([s_loc, s_loc], axis=0)
        va = np.concatenate([np.ascontiguousarray(v_loc[:, :, i].T)
                             .astype(np.float16) for i in range(3)], axis=1)
        m["v2da"] = np.concatenate([va, va], axis=0)          # [128, 3*BC]
        in_maps.append(m)
    return in_maps


def kernel(**inputs):
    from concourse.bass_utils import run_bass_kernel_spmd

    w = _host_prep(inputs)
    in_maps = _make_in_maps(inputs["x"], w)

    if "nc" not in _CACHE:
        _CACHE["nc"] = _build_program()
    nc = _CACHE["nc"]

    res = run_bass_kernel_spmd(nc, in_maps, core_ids=list(range(N_CORES)))

    out = np.empty((B, 256), dtype=np.float32)
    for c in range(N_CORES):
        o = res.results[c]["out"]                                # [256, BC]
        bs = slice(c * BC, (c + 1) * BC)
        out[bs, :64] = o[:64].T
        v = o[64:].reshape(3, 64, BC)
        out[bs, 64:] = v.transpose(2, 1, 0).reshape(BC, 192)
    return out
